# revision 29
# baseline (speedup 1.0000x reference)
"""Trainium2 Bass kernel for multi-head attention (B=8, N=1024, DM=512, H=8, D=64).

Sharding: data-parallel over batch — core i handles batch element i, weights
replicated, no collectives.

Per-core pipeline (all matmul operands float32r — tf32-like, 1 cyc/row):
  - host feeds current^T / hidden^T [512, 1024]
  - 10 consolidated input DMAs ordered by first use across both HWDGE
    queues (SP + ACT); projections emitted at (m-chunk, n-half)
    granularity so the PE starts as soon as the first chunks land
  - qT, kT = W^T @ x^T   (weights stationary)      [512 rows, 1024 tokens]
  - v natural [1024, 512] (hiddenT chunks stationary), stored ones-augmented
    as [128, 8, 65] per token-chunk (col 64 = 1.0 via memset -> softmax sums
    for free)
  - per (q-chunk, head-pair): dotsT [keys, queries], two heads packed
    (K=64 row tiling, banks 0/1 of a 2-bank PSUM tile); exp on ScalarE
    (no max subtraction -- logits are O(+-25), fp32 exp cannot overflow)
  - PV: out_h^T[65, 512] = v_aug^T @ expT, row 64 = softmax denominators
  - normalize with zero gather DMAs: DVE reciprocal in place on the PSUM
    denominator row, GpSimd partition-broadcast straight from that row,
    DVE multiplies read the PV PSUM directly; only the odd head needs one
    SBUF->SBUF DMA to cross into partitions 64:128
  - out-proj: out[t-chunk, :] = sum_ic oT[ic, t-chunk]^T @ Wo[ic, :]
"""
import sys

sys.path.insert(0, "/opt/trn_rl_repo")

import numpy as np

import concourse.bass as bass  # noqa: F401  (import keeps bass registered)
import concourse.mybir as mybir
import concourse.tile as tile
from concourse import bacc
from concourse.bass_utils import run_bass_kernel_spmd

F32 = mybir.dt.float32
F32R = mybir.dt.float32r

B, N, DM = 8, 1024, 512
H, D = 8, 64
NCORES = 8

_nc_cache = {}


def build_nc(loop_n=None):
    key = ("nc", loop_n)
    if key in _nc_cache:
        return _nc_cache[key]
    nc = bacc.Bacc("TRN2", target_bir_lowering=False, debug=False)

    curT_d = nc.dram_tensor("currentT", [DM, N], F32R, kind="ExternalInput").ap()
    hidT_d = nc.dram_tensor("hiddenT", [DM, N], F32R, kind="ExternalInput").ap()
    wq_d = nc.dram_tensor("Wq", [DM, H * D], F32R, kind="ExternalInput").ap()
    wkv_d = nc.dram_tensor("Wkv", [DM, 2 * H * D], F32R, kind="ExternalInput").ap()
    wo_d = nc.dram_tensor("Wo", [H * D, H * D], F32R, kind="ExternalInput").ap()
    out_d = nc.dram_tensor("out", [N, H * D], F32, kind="ExternalOutput").ap()

    with tile.TileContext(nc) as tc:
        if loop_n is None:
            build_body(nc, tc, curT_d, hidT_d, wq_d, wkv_d, wo_d, out_d)
        else:
            with tc.For_i(0, loop_n, 1):
                build_body(nc, tc, curT_d, hidT_d, wq_d, wkv_d, wo_d, out_d)
    nc.compile()
    _nc_cache[key] = nc
    return nc


def build_body(nc, tc, curT_d, hidT_d, wq_d, wkv_d, wo_d, out_d):
    import contextlib

    ctx = contextlib.ExitStack()
    with ctx:
        # ---------- pools ----------
        wpool = ctx.enter_context(tc.tile_pool(name="weights", bufs=1))
        actpool = ctx.enter_context(tc.tile_pool(name="acts", bufs=1))
        qkpool = ctx.enter_context(tc.tile_pool(name="qk", bufs=1))
        vpool = ctx.enter_context(tc.tile_pool(name="vaug", bufs=1))
        opool = ctx.enter_context(tc.tile_pool(name="ot", bufs=1))
        epool = ctx.enter_context(tc.tile_pool(name="expT", bufs=11))
        pvsb = ctx.enter_context(tc.tile_pool(name="pvsb", bufs=2))
        stgpool = ctx.enter_context(tc.tile_pool(name="stg", bufs=2))
        rrpool = ctx.enter_context(tc.tile_pool(name="rrow", bufs=1))
        bcpool = ctx.enter_context(tc.tile_pool(name="bcast", bufs=2))
        outsb = ctx.enter_context(tc.tile_pool(name="outsb", bufs=2))
        pvps = ctx.enter_context(tc.tile_pool(name="pvps", bufs=2, space="PSUM"))
        dpsum_cm = tc.tile_pool(name="dpsum", bufs=2, space="PSUM")
        dpsum = dpsum_cm.__enter__()
        ppsum_cm = tc.tile_pool(name="ppsum", bufs=2, space="PSUM")
        ppsum = ppsum_cm.__enter__()
        state = {"opps": None, "ppsum_open": True}

        # ---------- persistent tensors ----------
        wq_t = wpool.tile([128, 4, 512], F32R, tag="wq", name="wq")
        wkv_t = wpool.tile([128, 4, 1024], F32R, tag="wkv", name="wkv")
        wo_t = wpool.tile([128, 4, 512], F32R, tag="wo", name="wo")
        curT_t = actpool.tile([128, 4, 1024], F32R, tag="cur", name="cur")
        hidT_t = actpool.tile([128, 4, 1024], F32R, tag="hid", name="hid")
        qT = [qkpool.tile([128, 1024], F32R, tag=f"qT{m}", name=f"qT{m}") for m in range(4)]
        kT = [qkpool.tile([128, 1024], F32R, tag=f"kT{m}", name=f"kT{m}") for m in range(4)]
        vaug = [vpool.tile([128, H, D + 1], F32R, tag=f"va{t}", name=f"va{t}") for t in range(8)]
        oT = [opool.tile([128, 1024], F32R, tag=f"oT{i}", name=f"oT{i}") for i in range(4)]
        # pair 3's odd head stays in a partition-0-based tile (no stg DMA on
        # the tail's critical path); outproj splits its ic=3 accumulation
        # into two K=64 matmuls, with Wo's odd-block rows loaded at
        # partition 0 so fmap and weight share a start partition
        og3 = opool.tile([64, 1024], F32R, tag="og3", name="og3")
        wo3o = wpool.tile([64, 512], F32R, tag="wo3o", name="wo3o")

        # vaug softmax-denominator column: memset, no DMA, no dependencies
        # (bitcast: the memset ISA check rejects f32r, same bits as f32)
        for t in range(8):
            nc.gpsimd.memset(vaug[t][:, :, D:D + 1].bitcast(F32), 1.0)

        # ---------- input loads (10 DMAs, 2 HWDGE queues, first-use order) --
        # bus-serial arrival ~: wq0 .75 wk0 1.5 cur0 4.4 hid0 7.3 wv 10.2
        # hid1 13.1 cur1 16 wqr 18.2 wkr 20.4 wo 23.2 (us)
        q0, q1 = nc.sync, nc.scalar

        def r(dram_ap):
            return dram_ap.rearrange("(a p) c -> p a c", p=128)

        q0.dma_start(wq_t[:, :, 0:128], r(wq_d[:, 0:128]))
        q1.dma_start(wkv_t[:, :, 0:128], r(wkv_d[:, 0:128]))
        q0.dma_start(curT_t[:, 0:2, 0:512], r(curT_d[:, 0:512])[:, 0:2, :])
        q1.dma_start(curT_t[:, 2:4, 0:512], r(curT_d[:, 0:512])[:, 2:4, :])
        q0.dma_start(hidT_t[:, 0:2, 0:512], r(hidT_d[:, 0:512])[:, 0:2, :])
        q1.dma_start(hidT_t[:, 2:4, 0:512], r(hidT_d[:, 0:512])[:, 2:4, :])
        q0.dma_start(wkv_t[:, :, 512:1024], r(wkv_d[:, 512:1024]))
        q1.dma_start(hidT_t[:, :, 512:1024], r(hidT_d[:, 512:1024]))
        q0.dma_start(curT_t[:, :, 512:1024], r(curT_d[:, 512:1024]))
        q1.dma_start(wq_t[:, :, 128:512], r(wq_d[:, 128:512]))
        q0.dma_start(wkv_t[:, :, 128:512], r(wkv_d[:, 128:512]))
        q1.dma_start(wo_t[:], r(wo_d))
        q1.dma_start(wo3o[:], wo_d[448:512, :])

        # ---------- projection units ----------
        def emit_qproj(m, n2):
            ps = ppsum.tile([128, 512], F32, tag="proj", name="psq")
            for k in range(4):
                nc.tensor.matmul(
                    ps[:],
                    wq_t[:, k, m * 128:(m + 1) * 128],
                    curT_t[:, k, n2 * 512:(n2 + 1) * 512],
                    start=(k == 0), stop=(k == 3))
            nc.vector.tensor_copy(qT[m][:, n2 * 512:(n2 + 1) * 512], ps[:])

        def emit_kproj(m, n2):
            ps = ppsum.tile([128, 512], F32, tag="proj", name="psk")
            for k in range(4):
                nc.tensor.matmul(
                    ps[:],
                    wkv_t[:, k, m * 128:(m + 1) * 128],
                    hidT_t[:, k, n2 * 512:(n2 + 1) * 512],
                    start=(k == 0), stop=(k == 3))
            nc.vector.tensor_copy(kT[m][:, n2 * 512:(n2 + 1) * 512], ps[:])

        def emit_vproj(tc_i):
            ps = ppsum.tile([128, 512], F32, tag="proj", name="psv")
            for k in range(4):
                nc.tensor.matmul(
                    ps[:],
                    hidT_t[:, k, tc_i * 128:(tc_i + 1) * 128],
                    wkv_t[:, k, 512:1024],
                    start=(k == 0), stop=(k == 3))
            nc.vector.tensor_copy(
                vaug[tc_i][:, :, 0:D],
                ps[:].rearrange("p (h d) -> p h d", h=H))

        # ---------- attention helpers ----------
        def emit_dots_tile(js, kt):
            dp = dpsum.tile([128, 2, 512], F32, tag="dps", name="dps")
            qc, hp = js["qc"], js["hp"]
            nc.tensor.matmul(
                dp[:, 0, :],
                kT[hp][0:64, kt * 128:(kt + 1) * 128],
                qT[hp][0:64, qc * 512:(qc + 1) * 512],
                start=True, stop=True)
            nc.tensor.matmul(
                dp[:, 1, :],
                kT[hp][64:128, kt * 128:(kt + 1) * 128],
                qT[hp][64:128, qc * 512:(qc + 1) * 512],
                start=True, stop=True)
            nc.scalar.activation(
                js["etiles"][kt][:].rearrange("p a b -> p (a b)"),
                dp[:].rearrange("p a b -> p (a b)"),
                mybir.ActivationFunctionType.Exp)

        def emit_pv(js, kc):
            if kc == 0:
                js["pve"] = pvps.tile([D + 1, 512], F32, tag="pv", name="pve")
                js["pvo"] = pvps.tile([D + 1, 512], F32, tag="pv", name="pvo")
            et = js["etiles"][kc]
            hp = js["hp"]
            nc.tensor.matmul(js["pve"][:], vaug[kc][:, 2 * hp, :],
                             et[:, 0, :], start=(kc == 0), stop=(kc == 7))
            nc.tensor.matmul(js["pvo"][:], vaug[kc][:, 2 * hp + 1, :],
                             et[:, 1, :], start=(kc == 0), stop=(kc == 7))

        def emit_norm(js, stg_q=None):
            qc, hp = js["qc"], js["hp"]
            pve, pvo = js["pve"], js["pvo"]
            # 1/denominator in place on the PSUM row holding the sums, then
            # DMA it down to partition 0 (the Q7 broadcast ucode only reads
            # partition 0) — issued first so the broadcast chain starts
            # before the bulk copies occupy the DVE
            nc.vector.reciprocal(pve[D:D + 1, :], pve[D:D + 1, :])
            nc.vector.reciprocal(pvo[D:D + 1, :], pvo[D:D + 1, :])
            rrow_e = rrpool.tile([1, 512], F32, tag="rre", name="rre")
            nc.sync.dma_start(rrow_e[:], pve[D:D + 1, :])
            rrow_o = rrpool.tile([1, 512], F32, tag="rro", name="rro")
            (stg_q or nc.sync).dma_start(rrow_o[:], pvo[D:D + 1, :])
            # fast PSUM->SBUF copies so the PV banks free early — the next
            # job's PV would otherwise wait on the whole normalize chain
            psb_e = pvsb.tile([D + 1, 512], F32, tag="pvsb", name="psbe")
            nc.vector.tensor_copy(psb_e[0:D, :], pve[0:D, :])
            psb_o = pvsb.tile([D + 1, 512], F32, tag="pvsb", name="psbo")
            nc.vector.tensor_copy(psb_o[0:D, :], pvo[0:D, :])
            bc_e = bcpool.tile([64, 512], F32, tag="bc", name="bce")
            nc.gpsimd.partition_broadcast(bc_e[:], rrow_e[:])
            bc_o = bcpool.tile([64, 512], F32, tag="bc", name="bco")
            nc.gpsimd.partition_broadcast(bc_o[:], rrow_o[:])
            nc.vector.tensor_mul(
                oT[hp][0:64, qc * 512:(qc + 1) * 512], psb_e[0:D, :], bc_e[:])
            if hp == 3:
                nc.vector.tensor_mul(
                    og3[:, qc * 512:(qc + 1) * 512], psb_o[0:D, :], bc_o[:])
            else:
                # DVE lanes cannot cross partitions: compute in 0:64, then
                # DMA into partitions 64:128 of the oT tile.
                stg = stgpool.tile([64, 512], F32R, tag="stg", name="stg")
                nc.vector.tensor_mul(stg[:], psb_o[0:D, :], bc_o[:])
                (stg_q or nc.sync).dma_start(
                    oT[hp][64:128, qc * 512:(qc + 1) * 512], stg[:])

        def get_opps():
            if state["opps"] is None:
                state["opps"] = ctx.enter_context(
                    tc.tile_pool(name="opps", bufs=3, space="PSUM"))
            return state["opps"]

        def outproj_start(tc_i, n_ic):
            ops = get_opps().tile([128, 512], F32, tag="op", name="ops")
            for ic in range(n_ic):
                nc.tensor.matmul(
                    ops[:],
                    oT[ic][:, tc_i * 128:(tc_i + 1) * 128],
                    wo_t[:, ic, :],
                    start=(ic == 0), stop=False, skip_group_check=True)
            return ops

        def outproj_finish(tc_i, ops, n_ic, q=None, via_act=False):
            sl = slice(tc_i * 128, (tc_i + 1) * 128)
            for ic in range(n_ic, 3):
                nc.tensor.matmul(
                    ops[:], oT[ic][:, sl], wo_t[:, ic, :],
                    start=(ic == 0), stop=False, skip_group_check=True)
            nc.tensor.matmul(
                ops[:], oT[3][0:64, sl], wo_t[0:64, 3, :],
                start=False, stop=False, skip_group_check=True)
            nc.tensor.matmul(
                ops[:], og3[:, sl], wo3o[:],
                start=False, stop=True, skip_group_check=True)
            osb = outsb.tile([128, 512], F32, tag="osb", name="osb")
            if via_act:
                # tail copies go to the by-then-idle ACT engine so the DVE
                # queue is free for the last norm chain
                nc.scalar.activation(osb[:], ops[:],
                                     mybir.ActivationFunctionType.Copy)
            else:
                nc.vector.tensor_copy(osb[:], ops[:])
            (q or nc.sync).dma_start(
                out_d[tc_i * 128:(tc_i + 1) * 128, :], osb[:])

        def emit_outproj(qc):
            for t2 in range(4):
                tc_i = qc * 4 + t2
                ops = outproj_start(tc_i, 3)
                outproj_finish(tc_i, ops, 3)

        # ---------- schedule ----------
        # job 0 is special-cased around DMA arrival: its dots run in two
        # halves (kT[0] cols 0:512 land ~7.3us, cols 512:1024 ~13.3us) with
        # v-projections for token chunks 0:512 in between (wv lands ~10.2us).
        emit_qproj(0, 0)
        emit_kproj(0, 0)
        job0 = {"qc": 0, "hp": 0,
                "etiles": [epool.tile([128, 2, 512], F32R, tag="exp",
                                      name="exp") for _ in range(8)],
                "pve": None, "pvo": None}
        for kt in range(4):
            emit_dots_tile(job0, kt)
        for t in range(4):
            emit_vproj(t)
        emit_kproj(0, 1)
        for kt in range(4, 8):
            emit_dots_tile(job0, kt)
        emit_qproj(0, 1)

        # fillers for jobs 1+, budgeted per job so PE work stays balanced
        # against the ACT exp pace and each unit's data/consumer timing:
        # vp4-7 before pv(job0,kc4) runs in job1; qk[m] fully projected one
        # job before head-pair m's first dots job.
        job_fillers = [
            [lambda t=t: emit_vproj(t) for t in range(4, 8)]
            + [lambda: emit_qproj(1, 0), lambda: emit_kproj(1, 0),
               lambda: emit_qproj(1, 1), lambda: emit_kproj(1, 1)],
            [lambda: emit_qproj(2, 0), lambda: emit_kproj(2, 0)],
            [lambda: emit_qproj(2, 1), lambda: emit_kproj(2, 1)],
            [lambda: emit_qproj(3, 0), lambda: emit_kproj(3, 0)],
            [lambda: emit_qproj(3, 1), lambda: emit_kproj(3, 1)],
            [], [],
        ]

        # interleaved q-chunk order spreads projection fillers evenly:
        # head-pair m first needed at job 2m
        jobs = [(1, 0), (0, 1), (1, 1), (0, 2), (1, 2), (0, 3), (1, 3)]

        pending = job0
        for ji, (qc, hp) in enumerate(jobs):
            fillers = job_fillers[ji][::-1]
            cur = {"qc": qc, "hp": hp,
                   "etiles": [epool.tile([128, 2, 512], F32R, tag="exp",
                                         name="exp") for _ in range(8)],
                   "pve": None, "pvo": None}
            for kt in range(8):
                emit_dots_tile(cur, kt)
                # PV of the previous job overlaps this tile's ACT exp
                if pending is not None:
                    emit_pv(pending, kt)
                if fillers:
                    fillers.pop()()
            if ji == 4 and state["ppsum_open"]:
                state["ppsum_open"] = False
                ppsum_cm.__exit__(None, None, None)
            if pending is not None:
                emit_norm(pending)
            pending = cur
        # drain last job: the qc=0 out-projections interleave with the PV
        # drain (PV is paced by ACT's exps — outproj keeps PE busy in the
        # stall slots), the last norm's DVE chain starts right after the
        # last PV, and the qc=1 partials (ic 0-2: oT[0..2] + og3 halves
        # normalized earlier) overlap it; dots PSUM banks are dead by then —
        # release them so the outproj pool can hold 3 tiles
        dpsum_cm.__exit__(None, None, None)
        for kt in range(8):
            emit_pv(pending, kt)
            if kt in (3, 7):
                tc_i = kt // 4
                ops = outproj_start(tc_i, 3)
                outproj_finish(tc_i, ops, 3)
        emit_norm(pending, stg_q=nc.scalar)
        for tc_i in (2, 3):
            ops = outproj_start(tc_i, 3)
            outproj_finish(tc_i, ops, 3)
        part = [outproj_start(4 + t2, 3) for t2 in range(3)]
        for t2 in range(3):
            outproj_finish(4 + t2, part[t2], 3, via_act=(t2 % 2 == 0))
        ops = outproj_start(7, 3)
        outproj_finish(7, ops, 3, via_act=False)


def make_in_maps(inputs):
    current = np.asarray(inputs["current"], dtype=np.float32)
    hidden = np.asarray(inputs["hidden"], dtype=np.float32)
    Wq = np.ascontiguousarray(np.asarray(inputs["Wq"], dtype=np.float32))
    Wkv = np.ascontiguousarray(np.asarray(inputs["Wkv"], dtype=np.float32))
    Wo = np.ascontiguousarray(np.asarray(inputs["Wo"], dtype=np.float32))

    in_maps = []
    for i in range(NCORES):
        in_maps.append({
            "currentT": np.ascontiguousarray(current[i].T),
            "hiddenT": np.ascontiguousarray(hidden[i].T),
            "Wq": Wq, "Wkv": Wkv, "Wo": Wo,
        })
    return in_maps


def kernel(current, hidden, Wq, Wkv, Wo):
    in_maps = make_in_maps(
        {"current": current, "hidden": hidden, "Wq": Wq, "Wkv": Wkv, "Wo": Wo})
    nc = build_nc()
    res = run_bass_kernel_spmd(nc, in_maps, core_ids=list(range(NCORES)))
    out = np.stack([res.results[i]["out"] for i in range(NCORES)], axis=0)
    return out
